# revision 1
# baseline (speedup 1.0000x reference)
"""Gaussian-splat differentiable renderer on 8 TRN2 NeuronCores.

The reference renders N=4096 isotropic 2D gaussians into a 128x128 image
but returns only the first 1024 pixels (y in [0,8), x in [0,128)) per
batch.  The gaussians are isotropic and pixels live on a grid, so the
weight separates: w[n,(x,y)] = g(n,x) * f(n,y), g = exp(-((x-u)*sd)^2),
f = exp(-((y-v)*sd)^2), sd = sqrt(0.5)/scale.

Sharding: 8 cores = batch (2) x x-blocks of 32 columns (4).  Each core
holds all N gaussians (partition p, chunk k; n = p*32+k) and owns its 32
x-columns end to end -- no collectives.

Host folding (input-side only; cross-gaussian math stays on device):
R rows pre-scaled by (c*fx, c*fy, 1); per-gaussian products
pos_c*Rs[i,c] (+ t folded into the x term) so the camera transform is
two adds on device; iss = 1/scale; XI[k,x] = (x-cx)*c/s;
YI[y,k] = (y-cy)*c/s; opacity-premultiplied colors.

Device: cam = A+B+C; rz = 1/cam_z (reciprocal_approx_fast); VI then UI
muls (fp16 out); a custom fused DVE op SUB_SQ computes (t - uv)^2 in one
pass for both the f row and the four g groups, so ACT only runs exps;
T3[(d,y),k] = EFH*OC split 3/4 on DVE (2x fp16 mode) + 1/4 on GpSimd;
32 PSUM-accumulated fp16 matmuls chase the groups, with ~130 dummy
matmuls issued from t~7us to pull PE out of its 1.2GHz cold state before
the real stream; epilogue divides num by (den+eps).  eps = n_chunks*1e-8
>= 1.6e-7 > 1e-8 makes the reference's max(den, 1e-8) a no-op.
"""

import numpy as np

N_GAUSS = 4096
P = 128          # partitions
KC = 32          # gaussian chunks along the free axis (n = p*KC + k)
NX = 32          # x columns per core
NY = 8           # y rows in the output
N_CORES = 8
SQ2I = 0.7071067811865476
N_WARM_MM = 130

_BUILT = {}


def _quat2mat(q):
    q = q.astype(np.float32)
    q = q / np.float32(np.sqrt(np.float32((q * q).sum())))
    w, x, y, z = [np.float32(v) for v in q]
    return np.array(
        [
            [1 - 2 * (y * y + z * z), 2 * (x * y - z * w), 2 * (x * z + y * w)],
            [2 * (x * y + z * w), 1 - 2 * (x * x + z * z), 2 * (y * z - x * w)],
            [2 * (x * z - y * w), 2 * (y * z + x * w), 1 - 2 * (x * x + y * y)],
        ],
        np.float32,
    )


def _register_sub_sq():
    """Register a fused out = (in0 - in1)^2 custom DVE op (the sanctioned
    extension point per trainium-docs/custom-instructions/04-custom-dve-api:
    append to dve_ops.OPS).  Fusing the subtract and square into one DVE
    pass removes four ACT square passes from the critical path."""
    import concourse.dve_ops as dvo
    from concourse.dve_spec import Spec, Src0, Src1, sq, lower
    from concourse.dve_uop import DveOpSpec

    for op in dvo.OPS:
        if op.name == "SUB_SQ_ANT":
            return op
    spec = Spec(
        body=sq(Src0 - Src1),
        reference=lambda in0, in1, s0, s1, imm2: (
            (in0.astype(np.float32) - in1.astype(np.float32)) ** 2
        ),
    )
    op = dvo.DveOp("SUB_SQ_ANT", spec, subdim=False, uops_sha={})
    dvo.OPS.append(op)
    dvo._SUB_OPCODE_FOR_NAME[op.name] = dvo._CUSTOM_DVE_ROW_BASE + len(dvo.OPS) - 1
    dvo.CUSTOM_DVE_SPECS[op.name] = spec
    for ver in ("v3", "v4"):
        s = DveOpSpec(
            name=op.name,
            opcode=dvo.get_dve_sub_opcode(op.name),
            uops=lower(spec, ver=ver),
            rd1_en=True,
        )
        op.uops_sha[ver] = s.sha(ver)
    return op


def _build(eps):
    key = ("nc", float(eps))
    if key in _BUILT:
        return _BUILT[key]

    import concourse.mybir as mybir
    import concourse.tile as tile
    from concourse import bacc
    from concourse.tile_rust import add_dep_helper

    f32 = mybir.dt.float32
    f16 = mybir.dt.float16
    EXP = mybir.ActivationFunctionType.Exp
    SUB_SQ = _register_sub_sq()

    nc = bacc.Bacc("TRN2", target_bir_lowering=False, debug=False,
                   enable_asserts=False, num_devices=N_CORES)

    # rows 0-2: A_i = pos_x*Rs[i,0] + ts_i; 3-5: B_i = pos_y*Rs[i,1]
    ga1_d = nc.dram_tensor("ga1", [P, 6, KC], f32, kind="ExternalInput")
    # rows 0-2: C_i = pos_z*Rs[i,2]; row 3: iss = 1/scale
    ga2_d = nc.dram_tensor("ga2", [P, 4, KC], f32, kind="ExternalInput")
    xi_d = nc.dram_tensor("xi", [P, KC, NX], f16, kind="ExternalInput")
    # rows 0-7: YI[y,k]; rows 8-11: OC (opa*r, opa*g, opa*b, opa)
    yoc_d = nc.dram_tensor("yoc", [P, NY + 4, KC], f16, kind="ExternalInput")
    out_d = nc.dram_tensor("out", [NX, 24], f32, kind="ExternalOutput")

    with tile.TileContext(nc) as tc:
        with (
            tc.tile_pool(name="sb", bufs=1) as pool,
            tc.tile_pool(name="ps", bufs=1, space="PSUM") as psum,
        ):
            # PE warm-up: ~130 independent matmuls on a dummy tile keep the
            # PE activity window hot so the real stream runs at 2.4 GHz.
            DW = pool.tile([P, NX], f16)
            PSD = psum.tile([NX, NX], f32)
            nc.gpsimd.memset(DW[:], 0.25)
            for _ in range(N_WARM_MM):
                nc.tensor.matmul(PSD[:], DW[:], DW[:], start=True, stop=True)

            GA1 = pool.tile([P, 6, KC], f32)
            GA2 = pool.tile([P, 4, KC], f32)
            XI = pool.tile([P, KC, NX], f16)
            YOC = pool.tile([P, NY + 4, KC], f16)
            nc.sync.dma_start(GA1[:], ga1_d[:])
            nc.gpsimd.dma_start(GA2[:], ga2_d[:])
            nc.scalar.dma_start(YOC[:], yoc_d[:])
            nc.scalar.dma_start(XI[:], xi_d[:])

            YI = YOC[:, 0:NY, :]
            OC = YOC[:, NY : NY + 4, :]

            CAM = pool.tile([P, 3, KC], f32)
            RZM = pool.tile([P, 2, KC], f32)   # rz | m1 = rz*iss
            UIV = pool.tile([P, 2, KC], f16)   # ui | vi
            EFA = pool.tile([P, NY, KC], f16)
            EFH = pool.tile([P, NY, KC], f16)
            T3 = pool.tile([P, 4, NY, KC], f16)
            EGA = pool.tile([P, KC, NX], f16)
            EGH = pool.tile([P, KC, NX], f16)
            PS = psum.tile([NX, 32], f32)

            # camera: cam = A + B + C (t, c*fx, c*fy pre-folded)
            nc.vector.tensor_add(CAM[:], GA1[:, 0:3, :], GA1[:, 3:6, :])
            nc.vector.tensor_add(CAM[:], CAM[:], GA2[:, 0:3, :])
            nc.vector.reciprocal_approx_fast(RZM[:, 0, :], CAM[:, 2, :])
            nc.vector.tensor_mul(RZM[:, 1, :], RZM[:, 0, :], GA2[:, 3, :])
            # VI first: it unlocks the f path -> T3 -> all matmuls
            nc.vector.tensor_mul(UIV[:, 1, :], CAM[:, 1, :], RZM[:, 1, :])
            nc.vector.tensor_mul(UIV[:, 0, :], CAM[:, 0, :], RZM[:, 1, :])

            # f path: EFA = (YI - VI)^2 fused; exp on ACT; T3 split DVE/GpS
            with tc.high_priority():
                nc.vector._custom_dve(
                    SUB_SQ, out=EFA[:], in0=YI,
                    in1=UIV[:, 1, None, :].broadcast_to([P, NY, KC]),
                )
                nc.scalar.activation(
                    EFH[:].rearrange("p a b -> p (a b)"),
                    EFA[:].rearrange("p a b -> p (a b)"), EXP, scale=-1.0)
                # T3[p, d, y, k] = EFH[p, y, k] * OC[p, d, k]
                nc.gpsimd.tensor_mul(
                    T3[:, 3, :, :], EFH[:], OC[:, 3, None, :].broadcast_to(
                        [P, NY, KC]))
                t_dve = nc.vector.tensor_mul(
                    T3[:, 0:3, :, :],
                    EFH[:, None, :, :].broadcast_to([P, 3, NY, KC]),
                    OC[:, 0:3, None, :].broadcast_to([P, 3, NY, KC]),
                )

            # g path: fused (XI - UI)^2 per group on DVE; exp on ACT;
            # PE matmuls chase group by group.
            bounds = [0, 10, 20, 30, KC]
            for s in range(4):
                ks = slice(bounds[s], bounds[s + 1])
                GK = bounds[s + 1] - bounds[s]
                g_ss = nc.vector._custom_dve(
                    SUB_SQ, out=EGA[:, ks, :], in0=XI[:, ks, :],
                    in1=UIV[:, 0, ks, None].broadcast_to([P, GK, NX]),
                )
                if s == 1:
                    # T3's DVE share runs right after g0's sub-square
                    add_dep_helper(g_ss.ins, t_dve.ins, sync=False,
                                   reason="T3 before g1: unblock PE")
                nc.scalar.activation(
                    EGH[:, ks, :].rearrange("p a b -> p (a b)"),
                    EGA[:, ks, :].rearrange("p a b -> p (a b)"),
                    EXP, scale=-1.0,
                )
                for k in range(bounds[s], bounds[s + 1]):
                    nc.tensor.matmul(
                        PS[:], EGH[:, k, :],
                        T3[:, :, :, k].rearrange("x a b -> x (a b)"),
                        start=(k == 0), stop=(k == KC - 1),
                    )

            # img = num / (den + eps)
            DEN = pool.tile([NX, NY], f32)
            nc.vector.tensor_scalar_add(DEN[:], PS[:, 24:32], float(eps))
            REC = pool.tile([NX, NY], f32)
            nc.vector.reciprocal_approx_fast(REC[:], DEN[:])
            OUTT = pool.tile([NX, 3, NY], f32)
            nc.vector.tensor_mul(
                OUTT[:],
                PS[:, 0:24].rearrange("x (d y) -> x d y", y=NY),
                REC[:, None, :].broadcast_to([NX, 3, NY]),
            )
            nc.sync.dma_start(out_d[:], OUTT[:].rearrange("x d y -> x (d y)"))

    nc.compile()
    _BUILT[key] = nc
    return nc


def _core_inputs(core, positions, colors, opacities, scales, qvec, tvec,
                 intrinsics):
    b, xb = divmod(core, 4)
    R = _quat2mat(np.asarray(qvec, np.float32)[b])
    t = np.asarray(tvec, np.float32)[b]
    fx, fy, cx0, cy0 = np.asarray(intrinsics, np.float32)
    c = np.float32(SQ2I)

    row_scale = np.array([c * fx, c * fy, 1.0], np.float32)
    Rs = R * row_scale[:, None]
    ts = t * row_scale

    pos = np.asarray(positions, np.float32)          # [N, 3]
    px = pos[:, 0].reshape(P, KC)
    py = pos[:, 1].reshape(P, KC)
    pz = pos[:, 2].reshape(P, KC)
    iss = np.float32(1.0) / np.asarray(scales, np.float32).reshape(P, KC)

    ga1 = np.empty((P, 6, KC), np.float32)
    ga2 = np.empty((P, 4, KC), np.float32)
    for i in range(3):
        ga1[:, i, :] = px * Rs[i, 0] + ts[i]
        ga1[:, 3 + i, :] = py * Rs[i, 1]
        ga2[:, i, :] = pz * Rs[i, 2]
    ga2[:, 3, :] = iss

    isv = (c * iss).astype(np.float32)               # [P, KC]
    xs = (np.arange(NX, dtype=np.float32) + NX * xb - cx0)   # [NX]
    ys = (np.arange(NY, dtype=np.float32) - cy0)             # [NY]
    xi = (isv[:, :, None] * xs[None, None, :]).astype(np.float16)

    opa = np.asarray(opacities, np.float32).reshape(P, KC)
    col = np.asarray(colors, np.float32)
    yoc = np.empty((P, NY + 4, KC), np.float32)
    yoc[:, 0:NY, :] = ys[None, :, None] * isv[:, None, :]
    for i in range(3):
        yoc[:, NY + i, :] = opa * col[:, i].reshape(P, KC)
    yoc[:, NY + 3, :] = opa

    return {"ga1": ga1, "ga2": ga2, "xi": xi,
            "yoc": yoc.astype(np.float16)}


def kernel(positions, colors, opacities, scales, qvec, tvec, intrinsics,
           tile_hw, chunk_gauss, **run_kwargs):
    from concourse.bass_utils import run_bass_kernel_spmd

    tile_hw = int(tile_hw)
    chunk_gauss = int(chunk_gauss)
    assert tile_hw == 8 and positions.shape[0] == N_GAUSS
    n_chunks = -(-N_GAUSS // chunk_gauss)
    eps = np.float32(n_chunks * 1e-8)

    nc = _build(eps)
    in_maps = [
        _core_inputs(c, positions, colors, opacities, scales, qvec, tvec,
                     intrinsics)
        for c in range(N_CORES)
    ]
    res = run_bass_kernel_spmd(nc, in_maps, core_ids=list(range(N_CORES)),
                               **run_kwargs)

    B = np.asarray(qvec).shape[0]
    img = np.zeros((B, 3, NY, 128), np.float32)
    for c in range(N_CORES):
        b, xb = divmod(c, 4)
        o = res.results[c]["out"]               # [32x, 24 (ch*8+y)]
        img[b, :, :, xb * NX : (xb + 1) * NX] = o.T.reshape(3, NY, NX)
    out = img.reshape(B, 3, NY * 128).reshape(B, 3, 128, 8)
    kernel.last_results = res
    return out



# revision 2
# speedup vs baseline: 1.1018x; 1.1018x over previous
"""Gaussian-splat differentiable renderer on 8 TRN2 NeuronCores.

The reference renders N=4096 isotropic 2D gaussians into a 128x128 image
but returns only the first 1024 pixels (y in [0,8), x in [0,128)) per
batch.  The gaussians are isotropic and pixels live on a grid, so the
weight separates: w[n,(x,y)] = g(n,x) * f(n,y), g = exp(-((x-u)*sd)^2),
f = exp(-((y-v)*sd)^2), sd = sqrt(0.5)/scale.

Sharding: 8 cores = batch (2) x x-blocks of 32 columns (4).  Each core
holds all N gaussians (partition p, chunk k; n = p*32+k) and owns its 32
x-columns end to end -- no collectives.

Key device-side structure (v2):
 - exp(-d^2) is computed by the ACT engine's Derivative_Erf table
   (d/dx erf = 2/sqrt(pi) * exp(-x^2)); the 4/pi product factor is
   folded into the opacity-premultiplied colors on the host.  The DVE
   then only runs stock fp16 SUBTRACTs, which hit the 2-elem/cycle mode.
 - all tensors keep the gaussian-chunk axis k LAST so every broadcast
   (per-gaussian u/v against per-pixel grids) lands on a middle dim and
   the 2x DVE mode stays legal.
 - T3[(d,y),k] = EFH*OC entirely on DVE in two k-halves; 32
   PSUM-accumulated fp16 matmuls chase the two ACT g-halves; ~110 dummy
   matmuls warm the PE clock (HAM) before the real stream.
 - epilogue divides num by (den+eps).  eps = n_chunks*1e-8 >= 1.6e-7 >
   1e-8 makes the reference's max(den, 1e-8) a no-op.
"""

import numpy as np

N_GAUSS = 4096
P = 128          # partitions
KC = 32          # gaussian chunks along the free axis (n = p*KC + k)
KH = 16          # half of KC (ACT/matmul pipeline granularity)
NX = 32          # x columns per core
NY = 8           # y rows in the output
N_CORES = 8
SQ2I = 0.7071067811865476
PI4 = 0.7853981633974483   # pi/4, cancels the (2/sqrt(pi))^2 of D_Erf^2
N_WARM_MM = 110

_BUILT = {}


def _quat2mat(q):
    q = q.astype(np.float32)
    q = q / np.float32(np.sqrt(np.float32((q * q).sum())))
    w, x, y, z = [np.float32(v) for v in q]
    return np.array(
        [
            [1 - 2 * (y * y + z * z), 2 * (x * y - z * w), 2 * (x * z + y * w)],
            [2 * (x * y + z * w), 1 - 2 * (x * x + z * z), 2 * (y * z - x * w)],
            [2 * (x * z - y * w), 2 * (y * z + x * w), 1 - 2 * (x * x + y * y)],
        ],
        np.float32,
    )


def _build(eps):
    key = ("nc", float(eps))
    if key in _BUILT:
        return _BUILT[key]

    import concourse.mybir as mybir
    import concourse.tile as tile
    from concourse import bacc

    f32 = mybir.dt.float32
    f16 = mybir.dt.float16
    DERF = mybir.ActivationFunctionType.Derivative_Erf

    nc = bacc.Bacc("TRN2", target_bir_lowering=False, debug=False,
                   enable_asserts=False, num_devices=N_CORES)

    # rows: 0 camx*(c*fx), 1 camy*(c*fy), 2 camz, 3 iss=1/scale
    ga_d = nc.dram_tensor("ga", [P, 4, KC], f32, kind="ExternalInput")
    # rows 0-7: YI[y,k] = (y-cy)*c*iss; rows 8-11: OC' = (pi/4)*opa*(r,g,b,1)
    yoc_d = nc.dram_tensor("yoc", [P, NY + 4, KC], f16, kind="ExternalInput")
    # XI[p, x, k] = (x + 32*xb - cx)*c*iss[p,k]   (k LAST)
    xi_d = nc.dram_tensor("xi", [P, NX, KC], f16, kind="ExternalInput")
    out_d = nc.dram_tensor("out", [NX, 24], f32, kind="ExternalOutput")

    with tile.TileContext(nc) as tc:
        with (
            tc.tile_pool(name="sb", bufs=1) as pool,
            tc.tile_pool(name="ps", bufs=1, space="PSUM") as psum,
        ):
            # PE warm-up: independent matmuls keep the PE activity window
            # hot so the real stream runs at 2.4 GHz.
            DW = pool.tile([P, NX], f16)
            PSD = psum.tile([NX, NX], f32)
            nc.gpsimd.memset(DW[:], 0.25)
            for _ in range(N_WARM_MM):
                nc.tensor.matmul(PSD[:], DW[:], DW[:], start=True, stop=True)

            GA = pool.tile([P, 4, KC], f32)
            YOC = pool.tile([P, NY + 4, KC], f16)
            XI = pool.tile([P, NX, KC], f16)
            nc.sync.dma_start(GA[:], ga_d[:])
            nc.sync.dma_start(YOC[:], yoc_d[:])
            nc.gpsimd.dma_start(XI[:], xi_d[:])

            YI = YOC[:, 0:NY, :]
            OC = YOC[:, NY : NY + 4, :]

            RZM = pool.tile([P, 2, KC], f32)   # rz | m1 = rz*iss
            UIV = pool.tile([P, 2, KC], f16)   # ui | vi
            EFA = pool.tile([P, NY, KC], f16)
            EFH = pool.tile([P, NY, KC], f16)
            EGA = pool.tile([P, NX, KC], f16)
            EGH = pool.tile([P, NX, KC], f16)
            T3 = pool.tile([P, 4, NY, KC], f16)
            PS = psum.tile([NX, 32], f32)

            nc.vector.reciprocal_approx_fast(RZM[:, 0, :], GA[:, 2, :])
            nc.vector.tensor_mul(RZM[:, 1, :], RZM[:, 0, :], GA[:, 3, :])
            nc.vector.tensor_mul(
                UIV[:], GA[:, 0:2, :],
                RZM[:, 1, None, :].broadcast_to([P, 2, KC]))

            # f path: d = YI - VI (stock fp16 sub, 2x mode); ACT D_Erf
            nc.vector.tensor_sub(
                EFA[:], YI,
                UIV[:, 1, None, :].broadcast_to([P, NY, KC]))
            # g path halves: d = XI - UI (k-last keeps broadcasts mid-dim)
            for s in range(2):
                ks = slice(s * KH, (s + 1) * KH)
                nc.vector.tensor_sub(
                    EGA[:, :, ks], XI[:, :, ks],
                    UIV[:, 0, None, ks].broadcast_to([P, NX, KH]))

            nc.scalar.activation(
                EFH[:].rearrange("p a b -> p (a b)"),
                EFA[:].rearrange("p a b -> p (a b)"), DERF)
            for s in range(2):
                ks = slice(s * KH, (s + 1) * KH)
                nc.scalar.activation(EGH[:, :, ks], EGA[:, :, ks], DERF)

            # T3[p, d, y, k] = EFH[p, y, k] * OC[p, d, k], all on DVE (2x)
            for s in range(2):
                ks = slice(s * KH, (s + 1) * KH)
                nc.vector.tensor_mul(
                    T3[:, :, :, ks],
                    EFH[:, None, :, ks].broadcast_to([P, 4, NY, KH]),
                    OC[:, :, None, ks].broadcast_to([P, 4, NY, KH]),
                )

            for k in range(KC):
                nc.tensor.matmul(
                    PS[:], EGH[:, :, k],
                    T3[:, :, :, k].rearrange("x a b -> x (a b)"),
                    start=(k == 0), stop=(k == KC - 1),
                )

            # img = num / (den + eps)
            DEN = pool.tile([NX, NY], f32)
            nc.vector.tensor_scalar_add(DEN[:], PS[:, 24:32], float(eps))
            REC = pool.tile([NX, NY], f32)
            nc.vector.reciprocal_approx_fast(REC[:], DEN[:])
            OUTT = pool.tile([NX, 3, NY], f32)
            nc.vector.tensor_mul(
                OUTT[:],
                PS[:, 0:24].rearrange("x (d y) -> x d y", y=NY),
                REC[:, None, :].broadcast_to([NX, 3, NY]),
            )
            nc.sync.dma_start(out_d[:], OUTT[:].rearrange("x d y -> x (d y)"))

    nc.compile()
    _BUILT[key] = nc
    return nc


def _core_inputs(core, positions, colors, opacities, scales, qvec, tvec,
                 intrinsics):
    b, xb = divmod(core, 4)
    R = _quat2mat(np.asarray(qvec, np.float32)[b])
    t = np.asarray(tvec, np.float32)[b]
    fx, fy, cx0, cy0 = np.asarray(intrinsics, np.float32)
    c = np.float32(SQ2I)

    pos = np.asarray(positions, np.float32)          # [N, 3]
    px = pos[:, 0].reshape(P, KC)
    py = pos[:, 1].reshape(P, KC)
    pz = pos[:, 2].reshape(P, KC)
    iss = np.float32(1.0) / np.asarray(scales, np.float32).reshape(P, KC)

    camx = px * R[0, 0] + py * R[0, 1] + pz * R[0, 2] + t[0]
    camy = px * R[1, 0] + py * R[1, 1] + pz * R[1, 2] + t[1]
    camz = px * R[2, 0] + py * R[2, 1] + pz * R[2, 2] + t[2]

    ga = np.empty((P, 4, KC), np.float32)
    ga[:, 0, :] = camx * (c * fx)
    ga[:, 1, :] = camy * (c * fy)
    ga[:, 2, :] = camz
    ga[:, 3, :] = iss

    isv = (c * iss).astype(np.float32)               # [P, KC]
    xs = (np.arange(NX, dtype=np.float32) + NX * xb - cx0)   # [NX]
    ys = (np.arange(NY, dtype=np.float32) - cy0)             # [NY]
    xi = (xs[None, :, None] * isv[:, None, :]).astype(np.float16)

    opa4 = np.asarray(opacities, np.float32).reshape(P, KC) * np.float32(PI4)
    col = np.asarray(colors, np.float32)
    yoc = np.empty((P, NY + 4, KC), np.float32)
    yoc[:, 0:NY, :] = ys[None, :, None] * isv[:, None, :]
    for i in range(3):
        yoc[:, NY + i, :] = opa4 * col[:, i].reshape(P, KC)
    yoc[:, NY + 3, :] = opa4

    return {"ga": ga, "xi": xi, "yoc": yoc.astype(np.float16)}


def kernel(positions, colors, opacities, scales, qvec, tvec, intrinsics,
           tile_hw, chunk_gauss, **run_kwargs):
    from concourse.bass_utils import run_bass_kernel_spmd

    tile_hw = int(tile_hw)
    chunk_gauss = int(chunk_gauss)
    assert tile_hw == 8 and positions.shape[0] == N_GAUSS
    n_chunks = -(-N_GAUSS // chunk_gauss)
    eps = np.float32(n_chunks * 1e-8)

    nc = _build(eps)
    in_maps = [
        _core_inputs(c, positions, colors, opacities, scales, qvec, tvec,
                     intrinsics)
        for c in range(N_CORES)
    ]
    res = run_bass_kernel_spmd(nc, in_maps, core_ids=list(range(N_CORES)),
                               **run_kwargs)

    B = np.asarray(qvec).shape[0]
    img = np.zeros((B, 3, NY, 128), np.float32)
    for c in range(N_CORES):
        b, xb = divmod(c, 4)
        o = res.results[c]["out"]               # [32x, 24 (ch*8+y)]
        img[b, :, :, xb * NX : (xb + 1) * NX] = o.T.reshape(3, NY, NX)
    out = img.reshape(B, 3, NY * 128).reshape(B, 3, 128, 8)
    kernel.last_results = res
    return out
